# revision 12
# baseline (speedup 1.0000x reference)
"""Trainium2 Bass kernel for nn_DfDecoderStep (GRU decoder step + causal conv).

Data-parallel over batch across 8 NeuronCores (512 rows/core).  All
on-chip compute is feature-major (features on SBUF partitions, batch in
the free dim) so every matmul contracts along partitions with the
weights stationary.  The host does all layout work as part of sharding:
inputs arrive pre-transposed/tiled (so the kernel needs NO on-chip
transposes) and outputs are written feature-major and transposed back
on the host.  Matmuls run in float32r (single pass, full rate at free
dim >= 256; ~12-bit mantissa, rel err ~1e-4).  The conv buffer shift
(new_buf) is an exact fp32 passthrough of the input tiles.  Grouped
linears / grouped conv / pointwise conv are dense block-diagonal lhsT
matrices precomputed on the host.
"""

import numpy as np

import concourse.bacc as bacc
import concourse.tile as tile
from concourse import mybir
from concourse.bass_utils import run_bass_kernel_spmd

B, EMB, H, G, F, CH, O, OC, KT = 4096, 256, 256, 8, 96, 16, 5, 10, 5
BN_EPS = 1e-3
NCORES = 8
BS = B // NCORES          # 512 rows per core
NFB = 12                  # conv feature blocks (8 f x 10 (g,o) = 80 outputs each)
NCH = NFB * KT            # 60 conv input chunks of 128 features, (fb, t) order
NTH = 3                   # conv processed in thirds (SBUF budget)

f32 = mybir.dt.float32
f32r = mybir.dt.float32r
AF = mybir.ActivationFunctionType
OP = mybir.AluOpType

# which 128-wide cT tile each w_out output block contracts against
TAU = [(32 * ((80 * fb) // 120)) // 128 for fb in range(NFB)]


def _build_nc():
    nc = bacc.Bacc("TRN2")

    # ---- I/O (host supplies pre-transposed, tiled layouts) ----
    xt_d = nc.dram_tensor("xt", [128, NCH, BS], f32, kind="ExternalInput")
    embT_d = nc.dram_tensor("embT", [2, 128, BS], f32r, kind="ExternalInput")
    stT_d = nc.dram_tensor("stT", [4, 128, BS], f32r, kind="ExternalInput")
    winl_d = nc.dram_tensor("winl", [2, 128, 128], f32r, kind="ExternalInput")
    k0_d = nc.dram_tensor("k0", [H, 3 * H], f32r, kind="ExternalInput")
    rk0_d = nc.dram_tensor("rk0", [H, 3 * H], f32r, kind="ExternalInput")
    k1_d = nc.dram_tensor("k1", [H, 3 * H], f32r, kind="ExternalInput")
    rk1_d = nc.dram_tensor("rk1", [H, 3 * H], f32r, kind="ExternalInput")
    bz0_d = nc.dram_tensor("bz0", [4, 128, 1], f32, kind="ExternalInput")
    bxh0_d = nc.dram_tensor("bxh0", [2, 128, 1], f32, kind="ExternalInput")
    bhh0_d = nc.dram_tensor("bhh0", [2, 128, 1], f32, kind="ExternalInput")
    bz1_d = nc.dram_tensor("bz1", [4, 128, 1], f32, kind="ExternalInput")
    bxh1_d = nc.dram_tensor("bxh1", [2, 128, 1], f32, kind="ExternalInput")
    bhh1_d = nc.dram_tensor("bhh1", [2, 128, 1], f32, kind="ExternalInput")
    woutl_d = nc.dram_tensor("woutl", [NFB, 128, 80], f32r, kind="ExternalInput")
    convl_d = nc.dram_tensor("convl", [KT, 128, 80], f32r, kind="ExternalInput")
    pwl_d = nc.dram_tensor("pwl", [80, 80], f32r, kind="ExternalInput")
    bns_d = nc.dram_tensor("bns", [80, 1], f32, kind="ExternalInput")
    bnb_d = nc.dram_tensor("bnb", [80, 1], f32, kind="ExternalInput")
    fcw_d = nc.dram_tensor("fcw", [2, 128, 1], f32r, kind="ExternalInput")
    fcb_d = nc.dram_tensor("fcb", [1, 1], f32, kind="ExternalInput")

    coefsT_d = nc.dram_tensor("coefsT", [NFB, 80, BS], f32r, kind="ExternalOutput")
    alpha_d = nc.dram_tensor("alpha", [BS, 1], f32, kind="ExternalOutput")
    stoutT_d = nc.dram_tensor("stoutT", [4, 128, BS], f32r, kind="ExternalOutput")
    bufoutT_d = nc.dram_tensor(
        "bufoutT", [NFB, KT - 1, 128, BS], f32, kind="ExternalOutput"
    )

    with tile.TileContext(nc) as tc:
        with (
            tc.tile_pool(name="singles", bufs=1) as singles,
            tc.tile_pool(name="gwp", bufs=2) as gwp,
            tc.tile_pool(name="featp", bufs=1) as featp,
            tc.tile_pool(name="gwork", bufs=8) as gwork,
            tc.tile_pool(name="xg", bufs=2) as xgp,
            tc.tile_pool(name="xr", bufs=6) as xrp,
            tc.tile_pool(name="wk", bufs=3) as wk,
            tc.tile_pool(name="ps", bufs=8, space="PSUM") as psp,
        ):
            # ---------- weights / constants ----------
            winl = singles.tile([128, 2, 128], f32r, tag="winl")
            for c in range(2):
                nc.sync.dma_start(winl[:, c, :], winl_d[c])
            woutl = singles.tile([128, NFB, 80], f32r, tag="woutl")
            for j in range(NFB):
                nc.sync.dma_start(woutl[:, j, :], woutl_d[j])
            convl = singles.tile([128, KT, 80], f32r, tag="convl")
            for t in range(KT):
                nc.sync.dma_start(convl[:, t, :], convl_d[t])
            pwl = singles.tile([80, 80], f32r, tag="pwl")
            nc.sync.dma_start(pwl, pwl_d[:, :])
            bns = singles.tile([80, 1], f32, tag="bns")
            nc.sync.dma_start(bns, bns_d[:, :])
            bnb = singles.tile([80, 1], f32, tag="bnb")
            nc.sync.dma_start(bnb, bnb_d[:, :])
            fcw = singles.tile([128, 2], f32r, tag="fcw")
            for c in range(2):
                nc.sync.dma_start(fcw[:, c : c + 1], fcw_d[c])
            fcb = singles.tile([1, 1], f32, tag="fcb")
            nc.sync.dma_start(fcb, fcb_d[:, :])

            def bias_tile(dram, n, tag):
                t = singles.tile([128, n], f32, tag=tag, name=tag)
                for c in range(n):
                    nc.sync.dma_start(t[:, c : c + 1], dram[c])
                return t

            bz0 = bias_tile(bz0_d, 4, "bz0")
            bxh0 = bias_tile(bxh0_d, 2, "bxh0")
            bhh0 = bias_tile(bhh0_d, 2, "bhh0")
            bz1 = bias_tile(bz1_d, 4, "bz1")
            bxh1 = bias_tile(bxh1_d, 2, "bxh1")
            bhh1 = bias_tile(bhh1_d, 2, "bhh1")

            # ---------- persistent feature-major activations (fp32r) ----------
            embT = featp.tile([128, 2, BS], f32r, tag="embT")
            xinT = featp.tile([128, 2, BS], f32r, tag="xinT")
            h0T = featp.tile([128, 2, BS], f32r, tag="h0T")
            h1T = featp.tile([128, 2, BS], f32r, tag="h1T")
            o0T = featp.tile([128, 2, BS], f32r, tag="o0T")
            o1T = featp.tile([128, 2, BS], f32r, tag="o1T")
            cT = featp.tile([128, 2, BS], f32r, tag="cT")

            for c in range(2):
                nc.sync.dma_start(embT[:, c, :], embT_d[c])
            for tau in range(4):
                dst = h0T if tau < 2 else h1T
                nc.sync.dma_start(dst[:, tau % 2, :], stT_d[tau])

            # ---------- in-projection + relu ----------
            for c in range(2):
                ps = psp.tile([128, BS], f32, tag="ps", name=f"psin{c}")
                nc.tensor.matmul(
                    ps, winl[:, c, :], embT[:, c, :], start=True, stop=True
                )
                nc.vector.tensor_scalar_max(xinT[:, c, :], ps, 0.0)

            # ---------- GRU cells ----------
            def gru_cell(idx, xT, hT, kd, rkd, bz, bxh, bhh, outT):
                gk = gwp.tile([128, 2, 3 * H], f32r, tag="gw", name=f"gk{idx}")
                gr = gwp.tile([128, 2, 3 * H], f32r, tag="gw", name=f"gr{idx}")
                for kc in range(2):
                    nc.sync.dma_start(gk[:, kc, :], kd[kc * 128 : (kc + 1) * 128, :])
                    nc.sync.dma_start(
                        gr[:, kc, :], rkd[kc * 128 : (kc + 1) * 128, :]
                    )
                zs, rs = [], []
                for c in range(4):  # z gates: c 0,1; r gates: c 2,3
                    ps = psp.tile([128, BS], f32, tag="ps", name=f"pszr{idx}{c}")
                    msl = slice(c * 128, (c + 1) * 128)
                    nc.tensor.matmul(
                        ps, gk[:, 0, msl], xT[:, 0, :], start=True, stop=False
                    )
                    nc.tensor.matmul(
                        ps, gk[:, 1, msl], xT[:, 1, :], start=False, stop=False
                    )
                    nc.tensor.matmul(
                        ps, gr[:, 0, msl], hT[:, 0, :], start=False, stop=False
                    )
                    nc.tensor.matmul(
                        ps, gr[:, 1, msl], hT[:, 1, :], start=False, stop=True
                    )
                    g = gwork.tile([128, BS], f32, tag="gt", name=f"g{idx}{c}")
                    nc.scalar.activation(
                        g, ps, AF.Sigmoid, bias=bz[:, c : c + 1], scale=1.0
                    )
                    (zs if c < 2 else rs).append(g)
                for c in range(2):  # candidate + new h
                    msl = slice((4 + c) * 128, (5 + c) * 128)
                    psx = psp.tile([128, BS], f32, tag="ps", name=f"psx{idx}{c}")
                    nc.tensor.matmul(
                        psx, gk[:, 0, msl], xT[:, 0, :], start=True, stop=False
                    )
                    nc.tensor.matmul(
                        psx, gk[:, 1, msl], xT[:, 1, :], start=False, stop=True
                    )
                    psh = psp.tile([128, BS], f32, tag="ps", name=f"psh{idx}{c}")
                    nc.tensor.matmul(
                        psh, gr[:, 0, msl], hT[:, 0, :], start=True, stop=False
                    )
                    nc.tensor.matmul(
                        psh, gr[:, 1, msl], hT[:, 1, :], start=False, stop=True
                    )
                    t1 = gwork.tile([128, BS], f32, tag="gt", name=f"t1{idx}{c}")
                    nc.vector.scalar_tensor_tensor(
                        t1, psh, bhh[:, c : c + 1], rs[c], OP.add, OP.mult
                    )
                    t3 = gwork.tile([128, BS], f32, tag="gt", name=f"t3{idx}{c}")
                    nc.vector.tensor_add(t3, psx, t1)
                    cand = gwork.tile([128, BS], f32, tag="gt", name=f"cd{idx}{c}")
                    nc.scalar.activation(
                        cand, t3, AF.Tanh, bias=bxh[:, c : c + 1], scale=1.0
                    )
                    d = gwork.tile([128, BS], f32, tag="gt", name=f"d{idx}{c}")
                    nc.vector.tensor_sub(d, hT[:, c, :], cand)
                    e = gwork.tile([128, BS], f32, tag="gt", name=f"e{idx}{c}")
                    nc.vector.tensor_mul(e, d, zs[c])
                    nc.vector.tensor_add(outT[:, c, :], e, cand)

            gru_cell(0, xinT, h0T, k0_d, rk0_d, bz0, bxh0, bhh0, o0T)
            gru_cell(1, o0T, h1T, k1_d, rk1_d, bz1, bxh1, bhh1, o1T)

            # skip connection
            for c in range(2):
                nc.vector.tensor_add(cT[:, c, :], o1T[:, c, :], xinT[:, c, :])

            # ---------- new_state output (feature-major; host transposes) ----
            for c in range(2):
                nc.sync.dma_start(stoutT_d[c], o0T[:, c, :])
                nc.sync.dma_start(stoutT_d[2 + c], o1T[:, c, :])

            # ---------- alpha ----------
            psa = psp.tile([1, BS], f32, tag="ps", name="psa")
            nc.tensor.matmul(psa, fcw[:, 0:1], cT[:, 0, :], start=True, stop=False)
            nc.tensor.matmul(psa, fcw[:, 1:2], cT[:, 1, :], start=False, stop=True)
            al = singles.tile([1, BS], f32, tag="alpha")
            nc.scalar.activation(al, psa, AF.Sigmoid, bias=fcb[0:1, 0:1], scale=1.0)
            nc.sync.dma_start(alpha_d.rearrange("b one -> one b"), al)

            # ---------- conv path (no transposes: x arrives feature-major) --
            # xt chunks are (fb, t)-ordered: chunk fb*KT+t holds features
            # [(t*12+fb)*128, ...+128) of x = concat(buf, c0).
            ncast = 0
            for th in range(NTH):
                csl = slice(th * 20, (th + 1) * 20)
                xgt = xgp.tile([128, 20, BS], f32, tag="xg", name=f"xg{th}")
                nc.sync.dma_start(xgt, xt_d[:, csl, :])
                for fbl in range(4):
                    fb = th * 4 + fbl
                    # new_buf passthrough: frames t=1..4 of this fb block
                    nc.sync.dma_start(
                        bufoutT_d[fb].rearrange("t p b -> p t b"),
                        xgt[:, fbl * KT + 1 : (fbl + 1) * KT, :],
                    )
                    pcs = psp.tile([80, BS], f32, tag="ps", name=f"pcs{fb}")
                    for t in range(KT):
                        xr = xrp.tile([128, BS], f32r, tag="xr", name=f"xr{fb}_{t}")
                        # fp32r rounding pass, mostly on the idle GpSimd
                        ncast += 1
                        if ncast % 4:
                            nc.gpsimd.tensor_copy(xr, xgt[:, fbl * KT + t, :])
                        else:
                            nc.vector.tensor_copy(xr, xgt[:, fbl * KT + t, :])
                        nc.tensor.matmul(
                            pcs,
                            convl[:, t, :],
                            xr,
                            start=(t == 0),
                            stop=(t == KT - 1),
                        )
                    # drain: pw conv + bn/relu + w_out tanh + add -> coefsT
                    ysl = wk.tile([80, BS], f32r, tag="ys", name=f"ys{fb}")
                    nc.vector.tensor_copy(ysl, pcs)
                    pp = psp.tile([80, BS], f32, tag="ps", name=f"pp{fb}")
                    nc.tensor.matmul(pp, pwl, ysl, start=True, stop=True)
                    c0o = wk.tile([80, BS], f32, tag="c0o", name=f"c0o{fb}")
                    nc.scalar.activation(
                        c0o, pp, AF.Relu, bias=bnb[:, 0:1], scale=bns[:, 0:1]
                    )
                    psw = psp.tile([80, BS], f32, tag="ps", name=f"psw{fb}")
                    nc.tensor.matmul(
                        psw, woutl[:, fb, :], cT[:, TAU[fb], :], start=True, stop=True
                    )
                    tht = wk.tile([80, BS], f32, tag="tht", name=f"tht{fb}")
                    nc.scalar.activation(tht, psw, AF.Tanh, scale=1.0)
                    co = wk.tile([80, BS], f32r, tag="co", name=f"co{fb}")
                    nc.vector.tensor_add(co, c0o, tht)
                    nc.sync.dma_start(coefsT_d[fb], co)

    nc.finalize()
    return nc


_NC_CACHE = None


def _get_nc():
    global _NC_CACHE
    if _NC_CACHE is None:
        _NC_CACHE = _build_nc()
    return _NC_CACHE


def _rne11(x):
    """Round fp32 to float32r (11 explicit mantissa bits, round-nearest)."""
    b = np.ascontiguousarray(x, np.float32).view(np.uint32)
    shift = np.uint32(12)  # 23 - 11
    lsb = (b >> shift) & np.uint32(1)
    rounded = (b + np.uint32(0x7FF) + lsb) & np.uint32(0xFFFFF000)
    return rounded.view(np.float32)


def _prep_shared(w_in, gru_k0, gru_rk0, gru_b0, gru_k1, gru_rk1, gru_b1,
                 w_out, fc_a_w, fc_a_b, conv_w, pw_w,
                 bn_gamma, bn_beta, bn_mean, bn_var):
    f = np.float32
    shared = {}

    # dense block-diagonal in-projection lhsT: [2, 128, 128]
    winl = np.zeros((2, 128, 128), f)
    for g in range(G):
        c, gg = divmod(g, 4)
        winl[c, gg * 32 : (gg + 1) * 32, gg * 32 : (gg + 1) * 32] = w_in[g]
    shared["winl"] = _rne11(winl)

    shared["k0"] = _rne11(gru_k0)
    shared["rk0"] = _rne11(gru_rk0)
    shared["k1"] = _rne11(gru_k1)
    shared["rk1"] = _rne11(gru_rk1)

    for nm, b in (("0", gru_b0), ("1", gru_b1)):
        bz = (b[0, : 2 * H] + b[1, : 2 * H]).astype(f)
        shared["bz" + nm] = bz.reshape(4, 128, 1)
        shared["bxh" + nm] = b[0, 2 * H :].astype(f).reshape(2, 128, 1)
        shared["bhh" + nm] = b[1, 2 * H :].astype(f).reshape(2, 128, 1)

    # w_out grouped-linear as per-feature-block lhsT: [12, 128, 80]
    woutl = np.zeros((NFB, 128, 80), f)
    for fb in range(NFB):
        for j in range(80):
            feat = 80 * fb + j
            gw, og = divmod(feat, 120)
            r0 = 32 * gw - 128 * TAU[fb]
            woutl[fb, r0 : r0 + 32, j] = w_out[gw, :, og]
    shared["woutl"] = _rne11(woutl)

    # conv lhsT per time step: [5, 128, 80]
    wg = conv_w.reshape(KT, CH // 2, 2, OC // 2)  # [t, i, g, o]
    convl = np.zeros((KT, 128, 80), f)
    for t in range(KT):
        for fs in range(8):
            for g in range(2):
                for i in range(8):
                    k = fs * 16 + g * 8 + i
                    for o in range(OC // 2):
                        convl[t, k, fs * 10 + g * 5 + o] = wg[t, i, g, o]
    shared["convl"] = _rne11(convl)

    # pointwise conv lhsT: block-diag of pw_w over 8 f values
    pwl = np.zeros((80, 80), f)
    for fs in range(8):
        pwl[fs * 10 : (fs + 1) * 10, fs * 10 : (fs + 1) * 10] = pw_w
    shared["pwl"] = _rne11(pwl)

    inv = 1.0 / np.sqrt(bn_var.astype(np.float64) + BN_EPS)
    s10 = (inv * bn_gamma).astype(f)
    b10 = (bn_beta - bn_mean * inv * bn_gamma).astype(f)
    shared["bns"] = np.tile(s10, 8).reshape(80, 1)
    shared["bnb"] = np.tile(b10, 8).reshape(80, 1)

    shared["fcw"] = _rne11(fc_a_w).reshape(2, 128, 1)
    shared["fcb"] = np.asarray(fc_a_b, f).reshape(1, 1)
    return shared


def _make_in_maps(emb, c0, df_dec_state, df_convp_buf,
                  w_in, gru_k0, gru_rk0, gru_b0, gru_k1, gru_rk1, gru_b1,
                  w_out, fc_a_w, fc_a_b, conv_w, pw_w,
                  bn_gamma, bn_beta, bn_mean, bn_var):
    shared = _prep_shared(
        np.asarray(w_in, np.float32), np.asarray(gru_k0, np.float32),
        np.asarray(gru_rk0, np.float32), np.asarray(gru_b0, np.float32),
        np.asarray(gru_k1, np.float32), np.asarray(gru_rk1, np.float32),
        np.asarray(gru_b1, np.float32), np.asarray(w_out, np.float32),
        np.asarray(fc_a_w, np.float32), np.asarray(fc_a_b, np.float32),
        np.asarray(conv_w, np.float32), np.asarray(pw_w, np.float32),
        np.asarray(bn_gamma, np.float32), np.asarray(bn_beta, np.float32),
        np.asarray(bn_mean, np.float32), np.asarray(bn_var, np.float32),
    )
    emb = np.ascontiguousarray(emb, np.float32)
    c0 = np.ascontiguousarray(c0, np.float32)
    st = np.ascontiguousarray(df_dec_state, np.float32)
    buf = np.ascontiguousarray(df_convp_buf, np.float32)

    # host-side layout: feature-major tiled views (sharding glue)
    x = np.concatenate(
        [buf.reshape(B, KT - 1, F * CH), c0.reshape(B, 1, F * CH)], 1
    )  # [B, KT, 1536]
    # -> [128p, (fb, t) chunk, b]
    xt = x.reshape(B, KT, NFB, 128).transpose(3, 2, 1, 0).reshape(128, NCH, B)
    embT = emb.reshape(B, 2, 128).transpose(1, 2, 0)   # [2, 128, B]
    stT = st.reshape(B, 4, 128).transpose(1, 2, 0)     # [4, 128, B]

    in_maps = []
    for i in range(NCORES):
        sl = slice(i * BS, (i + 1) * BS)
        m = dict(shared)
        m["xt"] = np.ascontiguousarray(xt[:, :, sl])
        m["embT"] = np.ascontiguousarray(embT[:, :, sl])
        m["stT"] = np.ascontiguousarray(stT[:, :, sl])
        in_maps.append(m)
    return in_maps


def _gather(res):

    coefs = np.empty((B, O, 1, F, 2), np.float32)
    alpha = np.empty((B, 1), np.float32)
    new_state = np.empty((B, 2 * H), np.float32)
    new_buf = np.empty((B, KT - 1, F, CH), np.float32)
    for i, r in enumerate(res):
        sl = slice(i * BS, (i + 1) * BS)
        cf = r["coefsT"].reshape(F * OC, BS).T.reshape(BS, F, O, 2)
        coefs[sl] = cf.transpose(0, 2, 1, 3).reshape(BS, O, 1, F, 2)
        alpha[sl] = r["alpha"]
        new_state[sl] = r["stoutT"].reshape(2 * H, BS).T
        new_buf[sl] = (
            r["bufoutT"].transpose(3, 1, 0, 2).reshape(BS, KT - 1, F, CH)
        )
    return coefs, alpha, new_state, new_buf


def kernel(**inputs):
    nc = _get_nc()
    in_maps = _make_in_maps(**inputs)
    res = run_bass_kernel_spmd(nc, in_maps, list(range(NCORES))).results
    return _gather(res)


# revision 13
# speedup vs baseline: 1.1888x; 1.1888x over previous
"""Trainium2 Bass kernel for nn_DfDecoderStep (GRU decoder step + causal conv).

Data-parallel over batch across 8 NeuronCores (512 rows/core).  All
on-chip compute is feature-major (features on SBUF partitions, batch in
the free dim) so every matmul contracts along partitions with the
weights stationary.  The host does all layout work as part of sharding:
inputs arrive pre-transposed/tiled (so the kernel needs NO on-chip
transposes) and outputs are written feature-major and transposed back
on the host.  Matmuls run in float32r (single pass, full rate at free
dim >= 256; ~12-bit mantissa, rel err ~1e-4).  The conv buffer shift
(new_buf) is an exact fp32 passthrough of the input tiles.  Grouped
linears / grouped conv / pointwise conv are dense block-diagonal lhsT
matrices precomputed on the host.
"""

import numpy as np

import concourse.bacc as bacc
import concourse.tile as tile
from concourse import mybir
from concourse.bass_utils import run_bass_kernel_spmd

B, EMB, H, G, F, CH, O, OC, KT = 4096, 256, 256, 8, 96, 16, 5, 10, 5
BN_EPS = 1e-3
NCORES = 8
BS = B // NCORES          # 512 rows per core
NFB = 12                  # conv feature blocks (8 f x 10 (g,o) = 80 outputs each)
NCH = NFB * KT            # 60 conv input chunks of 128 features, (fb, t) order
NTH = 3                   # conv processed in thirds (SBUF budget)

f32 = mybir.dt.float32
f32r = mybir.dt.float32r
AF = mybir.ActivationFunctionType
OP = mybir.AluOpType

# which 128-wide cT tile each w_out output block contracts against
TAU = [(32 * ((80 * fb) // 120)) // 128 for fb in range(NFB)]


def _build_nc():
    nc = bacc.Bacc("TRN2")

    # ---- I/O (host supplies pre-transposed, tiled layouts) ----
    xt_d = nc.dram_tensor("xt", [128, NCH, BS], f32, kind="ExternalInput")
    embT_d = nc.dram_tensor("embT", [2, 128, BS], f32r, kind="ExternalInput")
    stT_d = nc.dram_tensor("stT", [4, 128, BS], f32r, kind="ExternalInput")
    winl_d = nc.dram_tensor("winl", [2, 128, 128], f32r, kind="ExternalInput")
    k0_d = nc.dram_tensor("k0", [H, 3 * H], f32r, kind="ExternalInput")
    rk0_d = nc.dram_tensor("rk0", [H, 3 * H], f32r, kind="ExternalInput")
    k1_d = nc.dram_tensor("k1", [H, 3 * H], f32r, kind="ExternalInput")
    rk1_d = nc.dram_tensor("rk1", [H, 3 * H], f32r, kind="ExternalInput")
    bz0_d = nc.dram_tensor("bz0", [4, 128, 1], f32, kind="ExternalInput")
    bxh0_d = nc.dram_tensor("bxh0", [2, 128, 1], f32, kind="ExternalInput")
    bhh0_d = nc.dram_tensor("bhh0", [2, 128, 1], f32, kind="ExternalInput")
    bz1_d = nc.dram_tensor("bz1", [4, 128, 1], f32, kind="ExternalInput")
    bxh1_d = nc.dram_tensor("bxh1", [2, 128, 1], f32, kind="ExternalInput")
    bhh1_d = nc.dram_tensor("bhh1", [2, 128, 1], f32, kind="ExternalInput")
    woutl_d = nc.dram_tensor("woutl", [NFB, 128, 80], f32r, kind="ExternalInput")
    convl_d = nc.dram_tensor("convl", [KT, 128, 80], f32, kind="ExternalInput")
    pwl_d = nc.dram_tensor("pwl", [80, 80], f32r, kind="ExternalInput")
    bns_d = nc.dram_tensor("bns", [80, 1], f32, kind="ExternalInput")
    bnb_d = nc.dram_tensor("bnb", [80, 1], f32, kind="ExternalInput")
    fcw_d = nc.dram_tensor("fcw", [2, 128, 1], f32r, kind="ExternalInput")
    fcb_d = nc.dram_tensor("fcb", [1, 1], f32, kind="ExternalInput")

    coefsT_d = nc.dram_tensor("coefsT", [NFB, 80, BS], f32r, kind="ExternalOutput")
    alpha_d = nc.dram_tensor("alpha", [BS, 1], f32, kind="ExternalOutput")
    stoutT_d = nc.dram_tensor("stoutT", [4, 128, BS], f32r, kind="ExternalOutput")
    bufoutT_d = nc.dram_tensor(
        "bufoutT", [NFB, KT - 1, 128, BS], f32, kind="ExternalOutput"
    )

    with tile.TileContext(nc) as tc:
        with (
            tc.tile_pool(name="singles", bufs=1) as singles,
            tc.tile_pool(name="gwp", bufs=2) as gwp,
            tc.tile_pool(name="featp", bufs=1) as featp,
            tc.tile_pool(name="gwork", bufs=8) as gwork,
            tc.tile_pool(name="xg", bufs=2) as xgp,
            tc.tile_pool(name="wk", bufs=3) as wk,
            tc.tile_pool(name="ps", bufs=8, space="PSUM") as psp,
        ):
            # ---------- weights / constants ----------
            winl = singles.tile([128, 2, 128], f32r, tag="winl")
            for c in range(2):
                nc.sync.dma_start(winl[:, c, :], winl_d[c])
            woutl = singles.tile([128, NFB, 80], f32r, tag="woutl")
            for j in range(NFB):
                nc.sync.dma_start(woutl[:, j, :], woutl_d[j])
            convl = singles.tile([128, KT, 80], f32, tag="convl")
            for t in range(KT):
                nc.sync.dma_start(convl[:, t, :], convl_d[t])
            pwl = singles.tile([80, 80], f32r, tag="pwl")
            nc.sync.dma_start(pwl, pwl_d[:, :])
            bns = singles.tile([80, 1], f32, tag="bns")
            nc.sync.dma_start(bns, bns_d[:, :])
            bnb = singles.tile([80, 1], f32, tag="bnb")
            nc.sync.dma_start(bnb, bnb_d[:, :])
            fcw = singles.tile([128, 2], f32r, tag="fcw")
            for c in range(2):
                nc.sync.dma_start(fcw[:, c : c + 1], fcw_d[c])
            fcb = singles.tile([1, 1], f32, tag="fcb")
            nc.sync.dma_start(fcb, fcb_d[:, :])

            def bias_tile(dram, n, tag):
                t = singles.tile([128, n], f32, tag=tag, name=tag)
                for c in range(n):
                    nc.sync.dma_start(t[:, c : c + 1], dram[c])
                return t

            bz0 = bias_tile(bz0_d, 4, "bz0")
            bxh0 = bias_tile(bxh0_d, 2, "bxh0")
            bhh0 = bias_tile(bhh0_d, 2, "bhh0")
            bz1 = bias_tile(bz1_d, 4, "bz1")
            bxh1 = bias_tile(bxh1_d, 2, "bxh1")
            bhh1 = bias_tile(bhh1_d, 2, "bhh1")

            # ---------- persistent feature-major activations (fp32r) ----------
            embT = featp.tile([128, 2, BS], f32r, tag="embT")
            xinT = featp.tile([128, 2, BS], f32r, tag="xinT")
            h0T = featp.tile([128, 2, BS], f32r, tag="h0T")
            h1T = featp.tile([128, 2, BS], f32r, tag="h1T")
            o0T = featp.tile([128, 2, BS], f32r, tag="o0T")
            o1T = featp.tile([128, 2, BS], f32r, tag="o1T")
            cT = featp.tile([128, 2, BS], f32r, tag="cT")

            for c in range(2):
                nc.sync.dma_start(embT[:, c, :], embT_d[c])
            for tau in range(4):
                dst = h0T if tau < 2 else h1T
                nc.sync.dma_start(dst[:, tau % 2, :], stT_d[tau])

            # ---------- in-projection + relu ----------
            for c in range(2):
                ps = psp.tile([128, BS], f32, tag="ps", name=f"psin{c}")
                nc.tensor.matmul(
                    ps, winl[:, c, :], embT[:, c, :], start=True, stop=True
                )
                nc.vector.tensor_scalar_max(xinT[:, c, :], ps, 0.0)

            # ---------- GRU cells ----------
            def gru_cell(idx, xT, hT, kd, rkd, bz, bxh, bhh, outT):
                gk = gwp.tile([128, 2, 3 * H], f32r, tag="gw", name=f"gk{idx}")
                gr = gwp.tile([128, 2, 3 * H], f32r, tag="gw", name=f"gr{idx}")
                for kc in range(2):
                    nc.sync.dma_start(gk[:, kc, :], kd[kc * 128 : (kc + 1) * 128, :])
                    nc.sync.dma_start(
                        gr[:, kc, :], rkd[kc * 128 : (kc + 1) * 128, :]
                    )
                zs, rs = [], []
                for c in range(4):  # z gates: c 0,1; r gates: c 2,3
                    ps = psp.tile([128, BS], f32, tag="ps", name=f"pszr{idx}{c}")
                    msl = slice(c * 128, (c + 1) * 128)
                    nc.tensor.matmul(
                        ps, gk[:, 0, msl], xT[:, 0, :], start=True, stop=False
                    )
                    nc.tensor.matmul(
                        ps, gk[:, 1, msl], xT[:, 1, :], start=False, stop=False
                    )
                    nc.tensor.matmul(
                        ps, gr[:, 0, msl], hT[:, 0, :], start=False, stop=False
                    )
                    nc.tensor.matmul(
                        ps, gr[:, 1, msl], hT[:, 1, :], start=False, stop=True
                    )
                    g = gwork.tile([128, BS], f32, tag="gt", name=f"g{idx}{c}")
                    nc.scalar.activation(
                        g, ps, AF.Sigmoid, bias=bz[:, c : c + 1], scale=1.0
                    )
                    (zs if c < 2 else rs).append(g)
                for c in range(2):  # candidate + new h
                    msl = slice((4 + c) * 128, (5 + c) * 128)
                    psx = psp.tile([128, BS], f32, tag="ps", name=f"psx{idx}{c}")
                    nc.tensor.matmul(
                        psx, gk[:, 0, msl], xT[:, 0, :], start=True, stop=False
                    )
                    nc.tensor.matmul(
                        psx, gk[:, 1, msl], xT[:, 1, :], start=False, stop=True
                    )
                    psh = psp.tile([128, BS], f32, tag="ps", name=f"psh{idx}{c}")
                    nc.tensor.matmul(
                        psh, gr[:, 0, msl], hT[:, 0, :], start=True, stop=False
                    )
                    nc.tensor.matmul(
                        psh, gr[:, 1, msl], hT[:, 1, :], start=False, stop=True
                    )
                    t1 = gwork.tile([128, BS], f32, tag="gt", name=f"t1{idx}{c}")
                    nc.vector.scalar_tensor_tensor(
                        t1, psh, bhh[:, c : c + 1], rs[c], OP.add, OP.mult
                    )
                    t3 = gwork.tile([128, BS], f32, tag="gt", name=f"t3{idx}{c}")
                    nc.vector.tensor_add(t3, psx, t1)
                    cand = gwork.tile([128, BS], f32, tag="gt", name=f"cd{idx}{c}")
                    nc.scalar.activation(
                        cand, t3, AF.Tanh, bias=bxh[:, c : c + 1], scale=1.0
                    )
                    d = gwork.tile([128, BS], f32, tag="gt", name=f"d{idx}{c}")
                    nc.vector.tensor_sub(d, hT[:, c, :], cand)
                    e = gwork.tile([128, BS], f32, tag="gt", name=f"e{idx}{c}")
                    nc.vector.tensor_mul(e, d, zs[c])
                    nc.vector.tensor_add(outT[:, c, :], e, cand)

            gru_cell(0, xinT, h0T, k0_d, rk0_d, bz0, bxh0, bhh0, o0T)
            gru_cell(1, o0T, h1T, k1_d, rk1_d, bz1, bxh1, bhh1, o1T)

            # skip connection
            for c in range(2):
                nc.vector.tensor_add(cT[:, c, :], o1T[:, c, :], xinT[:, c, :])

            # ---------- new_state output (feature-major; host transposes) ----
            for c in range(2):
                nc.sync.dma_start(stoutT_d[c], o0T[:, c, :])
                nc.sync.dma_start(stoutT_d[2 + c], o1T[:, c, :])

            # ---------- alpha ----------
            psa = psp.tile([1, BS], f32, tag="ps", name="psa")
            nc.tensor.matmul(psa, fcw[:, 0:1], cT[:, 0, :], start=True, stop=False)
            nc.tensor.matmul(psa, fcw[:, 1:2], cT[:, 1, :], start=False, stop=True)
            al = singles.tile([1, BS], f32, tag="alpha")
            nc.scalar.activation(al, psa, AF.Sigmoid, bias=fcb[0:1, 0:1], scale=1.0)
            nc.sync.dma_start(alpha_d.rearrange("b one -> one b"), al)

            # ---------- conv path (no transposes: x arrives feature-major) --
            # xt chunks are (fb, t)-ordered: chunk fb*KT+t holds features
            # [(t*12+fb)*128, ...+128) of x = concat(buf, c0).
            for th in range(NTH):
                csl = slice(th * 20, (th + 1) * 20)
                xgt = xgp.tile([128, 20, BS], f32, tag="xg", name=f"xg{th}")
                nc.sync.dma_start(xgt, xt_d[:, csl, :])
                for fbl in range(4):
                    fb = th * 4 + fbl
                    # new_buf passthrough: frames t=1..4 of this fb block
                    nc.sync.dma_start(
                        bufoutT_d[fb].rearrange("t p b -> p t b"),
                        xgt[:, fbl * KT + 1 : (fbl + 1) * KT, :],
                    )
                    pcs = psp.tile([80, BS], f32, tag="ps", name=f"pcs{fb}")
                    for t in range(KT):
                        nc.tensor.matmul(
                            pcs,
                            convl[:, t, :],
                            xgt[:, fbl * KT + t, :],
                            start=(t == 0),
                            stop=(t == KT - 1),
                        )
                    # drain: pw conv + bn/relu + w_out tanh + add -> coefsT
                    ysl = wk.tile([80, BS], f32r, tag="ys", name=f"ys{fb}")
                    nc.vector.tensor_copy(ysl, pcs)
                    pp = psp.tile([80, BS], f32, tag="ps", name=f"pp{fb}")
                    nc.tensor.matmul(pp, pwl, ysl, start=True, stop=True)
                    c0o = wk.tile([80, BS], f32, tag="c0o", name=f"c0o{fb}")
                    nc.scalar.activation(
                        c0o, pp, AF.Relu, bias=bnb[:, 0:1], scale=bns[:, 0:1]
                    )
                    psw = psp.tile([80, BS], f32, tag="ps", name=f"psw{fb}")
                    nc.tensor.matmul(
                        psw, woutl[:, fb, :], cT[:, TAU[fb], :], start=True, stop=True
                    )
                    tht = wk.tile([80, BS], f32, tag="tht", name=f"tht{fb}")
                    nc.scalar.activation(tht, psw, AF.Tanh, scale=1.0)
                    co = wk.tile([80, BS], f32r, tag="co", name=f"co{fb}")
                    nc.vector.tensor_add(co, c0o, tht)
                    nc.sync.dma_start(coefsT_d[fb], co)

    nc.finalize()
    return nc


_NC_CACHE = None


def _get_nc():
    global _NC_CACHE
    if _NC_CACHE is None:
        _NC_CACHE = _build_nc()
    return _NC_CACHE


def _rne11(x):
    """Round fp32 to float32r (11 explicit mantissa bits, round-nearest)."""
    b = np.ascontiguousarray(x, np.float32).view(np.uint32)
    shift = np.uint32(12)  # 23 - 11
    lsb = (b >> shift) & np.uint32(1)
    rounded = (b + np.uint32(0x7FF) + lsb) & np.uint32(0xFFFFF000)
    return rounded.view(np.float32)


def _prep_shared(w_in, gru_k0, gru_rk0, gru_b0, gru_k1, gru_rk1, gru_b1,
                 w_out, fc_a_w, fc_a_b, conv_w, pw_w,
                 bn_gamma, bn_beta, bn_mean, bn_var):
    f = np.float32
    shared = {}

    # dense block-diagonal in-projection lhsT: [2, 128, 128]
    winl = np.zeros((2, 128, 128), f)
    for g in range(G):
        c, gg = divmod(g, 4)
        winl[c, gg * 32 : (gg + 1) * 32, gg * 32 : (gg + 1) * 32] = w_in[g]
    shared["winl"] = _rne11(winl)

    shared["k0"] = _rne11(gru_k0)
    shared["rk0"] = _rne11(gru_rk0)
    shared["k1"] = _rne11(gru_k1)
    shared["rk1"] = _rne11(gru_rk1)

    for nm, b in (("0", gru_b0), ("1", gru_b1)):
        bz = (b[0, : 2 * H] + b[1, : 2 * H]).astype(f)
        shared["bz" + nm] = bz.reshape(4, 128, 1)
        shared["bxh" + nm] = b[0, 2 * H :].astype(f).reshape(2, 128, 1)
        shared["bhh" + nm] = b[1, 2 * H :].astype(f).reshape(2, 128, 1)

    # w_out grouped-linear as per-feature-block lhsT: [12, 128, 80]
    woutl = np.zeros((NFB, 128, 80), f)
    for fb in range(NFB):
        for j in range(80):
            feat = 80 * fb + j
            gw, og = divmod(feat, 120)
            r0 = 32 * gw - 128 * TAU[fb]
            woutl[fb, r0 : r0 + 32, j] = w_out[gw, :, og]
    shared["woutl"] = _rne11(woutl)

    # conv lhsT per time step: [5, 128, 80]
    wg = conv_w.reshape(KT, CH // 2, 2, OC // 2)  # [t, i, g, o]
    convl = np.zeros((KT, 128, 80), f)
    for t in range(KT):
        for fs in range(8):
            for g in range(2):
                for i in range(8):
                    k = fs * 16 + g * 8 + i
                    for o in range(OC // 2):
                        convl[t, k, fs * 10 + g * 5 + o] = wg[t, i, g, o]
    shared["convl"] = convl

    # pointwise conv lhsT: block-diag of pw_w over 8 f values
    pwl = np.zeros((80, 80), f)
    for fs in range(8):
        pwl[fs * 10 : (fs + 1) * 10, fs * 10 : (fs + 1) * 10] = pw_w
    shared["pwl"] = _rne11(pwl)

    inv = 1.0 / np.sqrt(bn_var.astype(np.float64) + BN_EPS)
    s10 = (inv * bn_gamma).astype(f)
    b10 = (bn_beta - bn_mean * inv * bn_gamma).astype(f)
    shared["bns"] = np.tile(s10, 8).reshape(80, 1)
    shared["bnb"] = np.tile(b10, 8).reshape(80, 1)

    shared["fcw"] = _rne11(fc_a_w).reshape(2, 128, 1)
    shared["fcb"] = np.asarray(fc_a_b, f).reshape(1, 1)
    return shared


def _make_in_maps(emb, c0, df_dec_state, df_convp_buf,
                  w_in, gru_k0, gru_rk0, gru_b0, gru_k1, gru_rk1, gru_b1,
                  w_out, fc_a_w, fc_a_b, conv_w, pw_w,
                  bn_gamma, bn_beta, bn_mean, bn_var):
    shared = _prep_shared(
        np.asarray(w_in, np.float32), np.asarray(gru_k0, np.float32),
        np.asarray(gru_rk0, np.float32), np.asarray(gru_b0, np.float32),
        np.asarray(gru_k1, np.float32), np.asarray(gru_rk1, np.float32),
        np.asarray(gru_b1, np.float32), np.asarray(w_out, np.float32),
        np.asarray(fc_a_w, np.float32), np.asarray(fc_a_b, np.float32),
        np.asarray(conv_w, np.float32), np.asarray(pw_w, np.float32),
        np.asarray(bn_gamma, np.float32), np.asarray(bn_beta, np.float32),
        np.asarray(bn_mean, np.float32), np.asarray(bn_var, np.float32),
    )
    emb = np.ascontiguousarray(emb, np.float32)
    c0 = np.ascontiguousarray(c0, np.float32)
    st = np.ascontiguousarray(df_dec_state, np.float32)
    buf = np.ascontiguousarray(df_convp_buf, np.float32)

    # host-side layout: feature-major tiled views (sharding glue)
    x = np.concatenate(
        [buf.reshape(B, KT - 1, F * CH), c0.reshape(B, 1, F * CH)], 1
    )  # [B, KT, 1536]
    # -> [128p, (fb, t) chunk, b]
    xt = x.reshape(B, KT, NFB, 128).transpose(3, 2, 1, 0).reshape(128, NCH, B)
    embT = emb.reshape(B, 2, 128).transpose(1, 2, 0)   # [2, 128, B]
    stT = st.reshape(B, 4, 128).transpose(1, 2, 0)     # [4, 128, B]

    in_maps = []
    for i in range(NCORES):
        sl = slice(i * BS, (i + 1) * BS)
        m = dict(shared)
        m["xt"] = np.ascontiguousarray(xt[:, :, sl])
        m["embT"] = np.ascontiguousarray(embT[:, :, sl])
        m["stT"] = np.ascontiguousarray(stT[:, :, sl])
        in_maps.append(m)
    return in_maps


def _gather(res):

    coefs = np.empty((B, O, 1, F, 2), np.float32)
    alpha = np.empty((B, 1), np.float32)
    new_state = np.empty((B, 2 * H), np.float32)
    new_buf = np.empty((B, KT - 1, F, CH), np.float32)
    for i, r in enumerate(res):
        sl = slice(i * BS, (i + 1) * BS)
        cf = r["coefsT"].reshape(F * OC, BS).T.reshape(BS, F, O, 2)
        coefs[sl] = cf.transpose(0, 2, 1, 3).reshape(BS, O, 1, F, 2)
        alpha[sl] = r["alpha"]
        new_state[sl] = r["stoutT"].reshape(2 * H, BS).T
        new_buf[sl] = (
            r["bufoutT"].transpose(3, 1, 0, 2).reshape(BS, KT - 1, F, CH)
        )
    return coefs, alpha, new_state, new_buf


def kernel(**inputs):
    nc = _get_nc()
    in_maps = _make_in_maps(**inputs)
    res = run_bass_kernel_spmd(nc, in_maps, list(range(NCORES))).results
    return _gather(res)
